# revision 6
# baseline (speedup 1.0000x reference)
"""Trainium2 Bass kernel for nn_Cy2Mixer_layer (gMLP block with conv/GCN/GCN
spatial mixers + fused output projection).

Sharding (8 cores):
  - The two GCN branches (sgu, cgu) + final projection/bias/residual are
    data-parallel over (B*T): 256 token-groups -> 32 per core, each a
    [N=128, D=256] tile (tokens on partitions).
  - The conv branch (tgu, Conv2d(T,T,(1,3)) channel mixer) needs full T per
    (b, n), so it is data-parallel over (B*N): 512 rows -> 64 per core,
    processed as 32 tiles of 2 rows ([2*T=128, D=256], tokens on partitions;
    the T-channel mix is a block-diagonal [128,128] matmul).
  Core outputs: og = xs/xc projections + b_out + residual (bt-sharded) and
  oc = xt projection (bn-sharded); the host scatters and adds the two.

All LayerNorm affine transforms are folded into the adjacent matmul weights
on the host; biases enter PSUM through low-rank matmuls (ones/colsum rows)
so the on-device elementwise work is minimal.
"""

import os
import sys
from contextlib import ExitStack

for _p in ("/opt/trn_rl_repo", "/root/.axon_site/_ro/trn_rl_repo"):
    if os.path.isdir(_p) and _p not in sys.path:
        sys.path.insert(0, _p)

import numpy as np

import bass_rust
import concourse.bass as bass
import concourse.tile as tile
from concourse import mybir
from concourse.bass_utils import run_bass_kernel_spmd

AF = mybir.ActivationFunctionType
ALU = mybir.AluOpType
F32 = mybir.dt.float32
F32R = mybir.dt.float32r
LN_EPS = 1e-5

B, T, N, D, F = 4, 64, 128, 256, 512
NCORES = 8
N_GCN = 32   # bt tiles per core
N_CONV = 32  # conv tiles per core (2 bn rows each)

_ctr = [0]


def _split_multi_waits(nc):
    """This walrus build rejects any instruction carrying >1 sync wait
    ("Too many sync wait commands"). Hoist all-but-one wait of every
    instruction onto dedicated same-engine NOPs inserted before it."""
    for f in nc.m.functions:
        for bb in f.blocks:
            insts = bb.instructions
            i = 0
            while i < len(insts):
                inst = insts[i]
                si = inst.sync_info
                if si is not None and si.on_wait is not None and len(si.on_wait) > 1:
                    waits = list(si.on_wait)
                    upd = list(si.on_update) if si.on_update is not None else []
                    inst.sync_info = bass_rust.SyncInfo(
                        on_wait=[waits[-1]], on_update=upd
                    )
                    for w in waits[:-1]:
                        _ctr[0] += 1
                        nop = mybir.InstNoOp(
                            name=f"wsplit-{_ctr[0]}",
                            sync_info=mybir.SyncInfo(on_wait=[w], on_update=[]),
                            bass_nofuse=True,
                            engine=inst.engine,
                        )
                        insts.insert(i, nop)
                        i += 1
                i += 1


def _r(ap):
    return ap.bitcast(F32R)


def _host_prep(inp):
    """Fold LN affines into weights; build matmul-ready constant layouts."""
    f32 = np.float32
    c = {}
    cir = np.asarray(inp["cirmat"])
    a = (cir != 0).astype(f32)
    np.fill_diagonal(a, 1.0)
    deg = a.sum(0).astype(f32)
    dinv = (1.0 / np.sqrt(deg)).astype(f32)
    a_hat = (a * dinv[:, None] * dinv[None, :]).astype(f32)
    c["a_hat"] = a_hat
    colsum = a_hat.sum(0).astype(f32)

    c["ident"] = np.eye(128, dtype=f32)
    c["eps_col"] = np.full((128, 1), LN_EPS, f32)
    c["ones_row"] = np.ones((1, 128), f32)

    w_out = np.asarray(inp["w_out"])
    bout = np.asarray(inp["b_out"]).astype(f32).copy()

    for p, pre, ng_, nb_, off in (
        ("s", "sgu", "n2_g", "n2_b", 256),
        ("c", "cgu", "n3_g", "n3_b", 512),
    ):
        ng = np.asarray(inp[f"{pre}_ng"])
        nb = np.asarray(inp[f"{pre}_nb"])
        w1 = np.asarray(inp[f"{pre}_w1"])
        b1 = np.asarray(inp[f"{pre}_b1"])
        sg = np.asarray(inp[f"{pre}_sg"])
        sb = np.asarray(inp[f"{pre}_sb"])
        gw = np.asarray(inp[f"{pre}_gw"])
        gb = np.asarray(inp[f"{pre}_gb"])
        w2 = np.asarray(inp[f"{pre}_w2"])
        b2 = np.asarray(inp[f"{pre}_b2"])
        c[f"{p}_w1p"] = np.ascontiguousarray((w1 * ng[None, :]).T).astype(f32)
        c[f"{p}_b1"] = (b1 + w1 @ nb)[None, :].astype(f32)
        c[f"{p}_grhs"] = np.ascontiguousarray((gw * sg[None, :]).T).astype(f32)
        c[f"{p}_bg_l"] = np.stack([colsum, np.ones(128, f32)]).astype(f32)
        c[f"{p}_bg_r"] = np.stack([gw @ sb, gb]).astype(f32)
        c[f"{p}_w2r"] = np.ascontiguousarray(w2.T).astype(f32)
        c[f"{p}_b2"] = b2[None, :].astype(f32)
        wsl = w_out[:, off : off + 256]
        c[f"{p}_wpr"] = np.ascontiguousarray((wsl * np.asarray(inp[ng_])[None, :]).T).astype(f32)
        bout = bout + wsl @ np.asarray(inp[nb_])

    ng = np.asarray(inp["tgu_ng"])
    nb = np.asarray(inp["tgu_nb"])
    w1 = np.asarray(inp["tgu_w1"])
    b1 = np.asarray(inp["tgu_b1"])
    sg = np.asarray(inp["tgu_sg"])
    sb = np.asarray(inp["tgu_sb"])
    cw = np.asarray(inp["tgu_cw"])[:, :, 0, :]  # [to, ti, dx]
    cb = np.asarray(inp["tgu_cb"])
    w2 = np.asarray(inp["tgu_w2"])
    b2 = np.asarray(inp["tgu_b2"])
    c["t_w1p"] = np.ascontiguousarray((w1 * ng[None, :]).T).astype(f32)
    c["t_b1"] = (b1 + w1 @ nb)[None, :].astype(f32)
    for dx in range(3):
        blk = np.zeros((128, 128), f32)
        lh = np.ascontiguousarray(cw[:, :, dx].T)  # [ti, to]
        blk[:64, :64] = lh
        blk[64:, 64:] = lh
        c[f"t_cw{dx}"] = blk
    # conv bias as a rank-4 matmul: cb + sum_dx cwsum_dx[to]*sb[fo+dx-1]
    cwsum = cw.sum(1)  # [to, dx]
    lhs = np.zeros((4, 128), f32)
    lhs[0] = np.concatenate([cb, cb])
    for dx in range(3):
        lhs[1 + dx] = np.concatenate([cwsum[:, dx], cwsum[:, dx]])
    rhs = np.zeros((4, 512), f32)
    rhs[0] = 1.0
    rhs[1, 1:] = sb[:511]   # dx=0 reads sb[fo-1]
    rhs[2] = sb             # dx=1 reads sb[fo]
    rhs[3, :511] = sb[1:]   # dx=2 reads sb[fo+1]
    c["t_cb_l"] = lhs
    c["t_cb_r"] = rhs
    c["t_w2r"] = np.ascontiguousarray(w2.T).astype(f32)
    c["t_b2"] = b2[None, :].astype(f32)
    wsl = w_out[:, 0:256]
    c["t_wpr"] = np.ascontiguousarray((wsl * np.asarray(inp["n1_g"])[None, :]).T).astype(f32)
    bout = bout + wsl @ np.asarray(inp["n1_b"])
    c["t_sg"] = np.broadcast_to(sg, (128, 512)).astype(f32).copy()
    c["bout"] = bout[None, :].astype(f32)
    return {k: np.ascontiguousarray(v, dtype=np.float32) for k, v in c.items()}


def _emit(nc, n_gcn, n_conv):
    xg = nc.dram_tensor("xg", [n_gcn * 128, 256], F32, kind="ExternalInput")
    xc = nc.dram_tensor("xc", [n_conv * 128, 256], F32, kind="ExternalInput")
    og = nc.dram_tensor("og", [n_gcn * 128, 256], F32, kind="ExternalOutput")
    oc = nc.dram_tensor("oc", [n_conv * 128, 256], F32, kind="ExternalOutput")

    wshapes = {
        "a_hat": [128, 128], "ident": [128, 128], "ones_row": [1, 128],
        "eps_col": [128, 1],
        "t_w1p": [256, 1024], "t_b1": [1, 1024],
        "t_cw0": [128, 128], "t_cw1": [128, 128], "t_cw2": [128, 128],
        "t_cb_l": [4, 128], "t_cb_r": [4, 512],
        "t_w2r": [512, 256], "t_b2": [1, 256], "t_wpr": [256, 256],
        "t_sg": [128, 512], "bout": [1, 256],
    }
    for p in ("s", "c"):
        wshapes.update({
            f"{p}_w1p": [256, 1024], f"{p}_b1": [1, 1024],
            f"{p}_grhs": [512, 512], f"{p}_bg_l": [2, 128], f"{p}_bg_r": [2, 512],
            f"{p}_w2r": [512, 256], f"{p}_b2": [1, 256], f"{p}_wpr": [256, 256],
        })
    wd = {k: nc.dram_tensor(k, v, F32, kind="ExternalInput") for k, v in wshapes.items()}

    with tile.TileContext(nc) as tc, ExitStack() as ctx:
        cpool = ctx.enter_context(tc.tile_pool(name="consts", bufs=1))
        wp = ctx.enter_context(tc.tile_pool(name="work", bufs=2))
        sp = ctx.enter_context(tc.tile_pool(name="stats", bufs=4))
        pp = ctx.enter_context(tc.tile_pool(name="ps", bufs=1, space="PSUM"))

        CS = {}
        _plain = {"ident", "eps_col", "t_sg"}
        for name, d in wd.items():
            pdim = d.shape[0]
            cast = (lambda ap: ap) if name in _plain else _r
            if pdim <= 128:
                t = cpool.tile(list(d.shape), F32, tag=name)
                nc.sync.dma_start(cast(t[:]), cast(d[:]))
                CS[name] = t
            else:
                ts = []
                for i in range(pdim // 128):
                    t = cpool.tile([128, d.shape[1]], F32, tag=f"{name}{i}")
                    nc.sync.dma_start(cast(t[:]), cast(d[i * 128 : (i + 1) * 128, :]))
                    ts.append(t)
                CS[name] = ts

        ident = CS["ident"]
        ones = CS["ones_row"]

        def ln_normalize(xin, width, tag, out_r=False):
            s6 = sp.tile([128, 6], F32, tag=f"s6{tag}")
            nc.vector.bn_stats(s6[:], xin)
            s2 = sp.tile([128, 2], F32, tag=f"s2{tag}")
            nc.vector.bn_aggr(s2[:], s6[:])
            std = sp.tile([128, 1], F32, tag=f"sd{tag}")
            nc.scalar.activation(std[:], s2[:, 1:2], AF.Sqrt, bias=CS["eps_col"][:])
            rstd = sp.tile([128, 1], F32, tag=f"rs{tag}")
            nc.vector.reciprocal(rstd[:], std[:])
            nm = sp.tile([128, 1], F32, tag=f"nm{tag}")
            nc.vector.tensor_scalar(
                nm[:], s2[:, 0:1], scalar1=rstd[:], scalar2=-1.0,
                op0=ALU.mult, op1=ALU.mult,
            )
            out = wp.tile([128, width], F32, tag=f"nrm{tag}")
            oap = _r(out[:]) if out_r else out[:]
            nc.scalar.activation(oap, xin, AF.Identity, bias=nm[:], scale=rstd[:])
            return out

        def transpose_to(xin, width, tag):
            tps = pp.tile([128, width], F32, tag="smallps" if width == 256 else "gtps")
            for cc in range(width // 128):
                sl = slice(cc * 128, (cc + 1) * 128)
                nc.tensor.transpose(tps[:, sl], xin[:, sl], ident[:])
            out = wp.tile([128, width], F32, tag=tag)
            nc.scalar.copy(_r(out[:]), tps[:])
            return out

        def load_front(xsrc, i):
            """DMA in + LN1 + transpose -> (X, xhT)."""
            X = wp.tile([128, 256], F32, tag="X")
            nc.sync.dma_start(X[:], xsrc[i * 128 : (i + 1) * 128, :])
            xhat = ln_normalize(X[:], 256, "ln1")
            xhT = transpose_to(xhat[:], 256, "xhT")
            return X, xhT

        def mlp_in(xhT, w1p, b1):
            """h = gelu(xhat @ w1p + b1) via PSUM accumulation."""
            h_ps = pp.tile([128, 1024], F32, tag="hps")
            for j in range(2):
                hj = h_ps[:, j * 512 : (j + 1) * 512]
                for cc in range(2):
                    sl = slice(cc * 128, (cc + 1) * 128)
                    nc.tensor.matmul(
                        hj, _r(xhT[:, sl]), _r(w1p[cc][:, j * 512 : (j + 1) * 512]),
                        start=(cc == 0), stop=False,
                    )
                nc.tensor.matmul(
                    hj, _r(ones[:]), _r(b1[:, j * 512 : (j + 1) * 512]),
                    start=False, stop=True, skip_group_check=True,
                )
            h = wp.tile([128, 1024], F32, tag="h")
            nc.scalar.activation(h[:], h_ps[:], AF.Gelu)
            return h

        def backend(gated_src, u, X, p, tag2):
            """gated = psum*u, transpose, w2 matmul + b2 + residual, LN -> xsh."""
            gated = wp.tile([128, 512], F32, tag="gated")
            nc.vector.tensor_tensor(gated[:], gated_src[:], u, op=ALU.mult)
            gT = transpose_to(gated[:], 512, "gT")
            blk_ps = pp.tile([128, 256], F32, tag="smallps")
            for fc in range(4):
                sl = slice(fc * 128, (fc + 1) * 128)
                nc.tensor.matmul(
                    blk_ps[:], _r(gT[:, sl]), _r(CS[f"{p}_w2r"][fc][:]),
                    start=(fc == 0), stop=False,
                )
            nc.tensor.matmul(
                blk_ps[:], _r(ones[:]), _r(CS[f"{p}_b2"][:]),
                start=False, stop=True, skip_group_check=True,
            )
            blk = wp.tile([128, 256], F32, tag="blk")
            nc.vector.scalar_tensor_tensor(
                blk[:], blk_ps[:], 0.0, X[:], op0=ALU.add, op1=ALU.add
            )
            xsh = ln_normalize(blk[:], 256, tag2)
            return transpose_to(xsh[:], 256, "xshT")

        # ---------------- GCN tiles ----------------
        for i in range(n_gcn):
            X, xhT = load_front(xg, i)
            fin_ps = pp.tile([128, 256], F32, tag="finps")
            for bi, p in enumerate(("s", "c")):
                h = mlp_in(xhT, CS[f"{p}_w1p"], CS[f"{p}_b1"][:])
                u = h[:, 0:512]
                v = h[:, 512:1024]
                vhat = ln_normalize(v, 512, "ln2", out_r=True)
                yt_ps = pp.tile([128, 512], F32, tag="ytps")
                for fc in range(4):
                    sl = slice(fc * 128, (fc + 1) * 128)
                    nc.tensor.matmul(
                        yt_ps[:, sl], _r(vhat[:, sl]), _r(CS["a_hat"][:]),
                        start=True, stop=True,
                    )
                yt = wp.tile([128, 512], F32, tag="yt")
                nc.scalar.copy(_r(yt[:]), yt_ps[:])
                g_ps = pp.tile([128, 512], F32, tag="gps")
                for fc in range(4):
                    sl = slice(fc * 128, (fc + 1) * 128)
                    nc.tensor.matmul(
                        g_ps[:], _r(yt[:, sl]), _r(CS[f"{p}_grhs"][fc][:]),
                        start=(fc == 0), stop=False,
                    )
                nc.tensor.matmul(
                    g_ps[:], _r(CS[f"{p}_bg_l"][:]), _r(CS[f"{p}_bg_r"][:]),
                    start=False, stop=True, skip_group_check=True,
                )
                xshT = backend(g_ps, u, X, p, "ln3")
                for cc in range(2):
                    sl = slice(cc * 128, (cc + 1) * 128)
                    nc.tensor.matmul(
                        fin_ps[:], _r(xshT[:, sl]), _r(CS[f"{p}_wpr"][cc][:]),
                        start=(bi == 0 and cc == 0), stop=False,
                        skip_group_check=True,
                    )
            nc.tensor.matmul(
                fin_ps[:], _r(ones[:]), _r(CS["bout"][:]),
                start=False, stop=True, skip_group_check=True,
            )
            outt = wp.tile([128, 256], F32, tag="outt")
            nc.vector.scalar_tensor_tensor(
                outt[:], fin_ps[:], 0.0, X[:], op0=ALU.add, op1=ALU.add
            )
            nc.sync.dma_start(og[i * 128 : (i + 1) * 128, :], outt[:])

        # ---------------- conv tiles ----------------
        for i in range(n_conv):
            X, xhT = load_front(xc, i)
            h = mlp_in(xhT, CS["t_w1p"], CS["t_b1"][:])
            u = h[:, 0:512]
            v = h[:, 512:1024]
            vhat = ln_normalize(v, 512, "ln2t")
            vs = wp.tile([128, 512], F32, tag="vs")
            nc.vector.tensor_tensor(_r(vs[:]), vhat[:], CS["t_sg"][:], op=ALU.mult)
            gc_ps = pp.tile([128, 512], F32, tag="gps")
            nc.tensor.matmul(gc_ps[:, 0:512], _r(CS["t_cw1"][:]), _r(vs[:, 0:512]),
                             start=True, stop=False)
            # shifted taps: fp32r needs even sizes + 8B-aligned dst, so do the
            # bulk as even-aligned fp32r and patch the edge column in fp32.
            nc.tensor.matmul(gc_ps[:, 2:512], _r(CS["t_cw0"][:]), _r(vs[:, 1:511]),
                             start=False, stop=False, skip_group_check=True)
            nc.tensor.matmul(gc_ps[:, 1:2], CS["t_cw0"][:], vs[:, 0:1],
                             start=False, stop=False, skip_group_check=True)
            nc.tensor.matmul(gc_ps[:, 0:510], _r(CS["t_cw2"][:]), _r(vs[:, 1:511]),
                             start=False, stop=False, skip_group_check=True)
            nc.tensor.matmul(gc_ps[:, 510:511], CS["t_cw2"][:], vs[:, 511:512],
                             start=False, stop=False, skip_group_check=True)
            nc.tensor.matmul(gc_ps[:, 0:512], _r(CS["t_cb_l"][:]), _r(CS["t_cb_r"][:]),
                             start=False, stop=True, skip_group_check=True)
            xshT = backend(gc_ps, u, X, "t", "ln3t")
            oc_ps = pp.tile([128, 256], F32, tag="smallps")
            for cc in range(2):
                sl = slice(cc * 128, (cc + 1) * 128)
                nc.tensor.matmul(
                    oc_ps[:], _r(xshT[:, sl]), _r(CS["t_wpr"][cc][:]),
                    start=(cc == 0), stop=(cc == 1), skip_group_check=True,
                )
            occ = wp.tile([128, 256], F32, tag="outt")
            nc.scalar.copy(occ[:], oc_ps[:])
            nc.sync.dma_start(oc[i * 128 : (i + 1) * 128, :], occ[:])


def build(n_gcn=N_GCN, n_conv=N_CONV):
    nc = bass.Bass()
    _emit(nc, n_gcn, n_conv)
    _split_multi_waits(nc)
    return nc


def kernel(**inputs):
    consts = _host_prep(inputs)
    x = np.ascontiguousarray(np.asarray(inputs["x"], dtype=np.float32))
    xg_full = x.reshape(B * T, N, D)
    xc_full = np.ascontiguousarray(x.transpose(0, 2, 1, 3)).reshape(B * N, T, D)

    nc = build()
    in_maps = []
    for k in range(NCORES):
        m = dict(consts)
        m["xg"] = np.ascontiguousarray(xg_full[32 * k : 32 * (k + 1)]).reshape(N_GCN * 128, 256)
        m["xc"] = np.ascontiguousarray(xc_full[64 * k : 64 * (k + 1)]).reshape(N_CONV * 128, 256)
        in_maps.append(m)
    trace = os.environ.get("BASS_KERNEL_TRACE") == "1"
    res = run_bass_kernel_spmd(nc, in_maps, core_ids=list(range(NCORES)), trace=trace)
    if trace and res.exec_time_ns is not None:
        print(f"HW exec time: {res.exec_time_ns} ns")
    kernel.last_result = res
    og_full = np.stack([r["og"] for r in res.results]).reshape(B * T, N, D).reshape(B, T, N, D)
    oc_full = (
        np.stack([r["oc"] for r in res.results])
        .reshape(B * N, T, D)
        .reshape(B, N, T, D)
        .transpose(0, 2, 1, 3)
    )
    return (og_full + oc_full).astype(np.float32)
